# revision 12
# baseline (speedup 1.0000x reference)
"""Trainium2 Bass kernel for nn_DeformableAlign (B=8, C=256, H=W=128).

Sharding: pure data parallel — batch dim across 8 NeuronCores, params
replicated, no cross-device comms.

Per-core pipeline (one batch item, channel-major [C, H*W] in DRAM):
  P1: stream padded channel-major row-slabs; fused depthwise3x3+1x1 offset
      head as 9 shifted matmuls on TensorE (fp32r, contraction 512x9);
      PE transposes build a pixel-major [HW, C] DRAM copy per map;
      global-avg-pool accumulated via ones-matvec on the pm tiles.
  P1.5: modulation MLP (matvecs + ACT relu/sigmoid); per-pixel sample
      coords/weights on DVE; gather indices int16 in the SWDGE wrapped
      layout via a pair of PE-transpose swizzles.
  P2: dma_gather of 2-pixel pairs (2KB descriptors) from the pm copy;
      4-term lerp with per-partition-scalar DVE ops; output transpose on
      PE with the modulation scale folded into the PSUM->SBUF ACT copy;
      quality = sigmoid(1 - mean_c |rgb-tir|) via abs + ones-matvec.
"""
import os
import sys
import time

import numpy as np

for _p in ("/opt/trn_rl_repo", "/root/.axon_site/_ro/trn_rl_repo"):
    if _p not in sys.path and os.path.isdir(_p):
        sys.path.insert(0, _p)

H = W = 128
HW = H * W
C = 256
C2 = 512
HID = 64
RP = 130          # padded row pitch (zero col each side)
NRC = 8           # output rows per phase-1 chunk
NBPX = 512        # pixels per gather batch (4 rows)

LAST_EXEC_NS = None
_CACHE = {}

_PARAM_SHAPES = {
    "w9T": [128, 2304], "bias_total": [64, 1], "WmT": [64, 4],
    "bm4": [128, 4], "m1T": [128, 256], "m1b": [64, 1],
    "m2T": [64, 256], "m2b": [128, 2], "iotaRow": [128, 128],
    "iotaCol": [128, 1], "I128": [128, 128], "ones_col": [128, 1],
}


def _fold_weights(inp):
    dw_w = np.asarray(inp['dw_w'], np.float32)
    dw_b = np.asarray(inp['dw_b'], np.float32)
    p1_w = np.asarray(inp['p1_w'], np.float32)
    p1_b = np.asarray(inp['p1_b'], np.float32)
    p2_w = np.asarray(inp['p2_w'], np.float32)
    p2_b = np.asarray(inp['p2_b'], np.float32)
    m1_w = np.asarray(inp['m1_w'], np.float32)
    m1_b = np.asarray(inp['m1_b'], np.float32)
    m2_w = np.asarray(inp['m2_w'], np.float32)
    m2_b = np.asarray(inp['m2_b'], np.float32)

    dw = dw_w.reshape(C2, 9)
    w9T = np.zeros((128, 36 * 64), np.float32)
    for b in range(4):
        for k in range(9):
            blk = p1_w[:, b * 128:(b + 1) * 128] * dw[b * 128:(b + 1) * 128, k][None, :]
            w9T[:, (b * 9 + k) * 64:(b * 9 + k + 1) * 64] = blk.T
    bias_total = (p1_w @ dw_b + p1_b).astype(np.float32).reshape(64, 1)

    groups = [[0, 2, 4, 6], [1, 3, 5, 7], [8, 10, 12, 14], [9, 11, 13, 15]]
    Wm = np.stack([6.35 * 0.25 * p2_w[g].sum(0) for g in groups])
    bm = np.array([6.35 * 0.25 * p2_b[g].sum() for g in groups], np.float32)
    WmT = np.ascontiguousarray(Wm.T, dtype=np.float32)
    bm4 = np.tile(bm.reshape(1, 4), (128, 1)).astype(np.float32)

    m1T = np.zeros((128, 4 * 64), np.float32)
    for b in range(4):
        m1T[:, b * 64:(b + 1) * 64] = m1_w[:, b * 128:(b + 1) * 128].T
    m2T = np.zeros((64, 2 * 128), np.float32)
    for cb in range(2):
        m2T[:, cb * 128:(cb + 1) * 128] = m2_w[cb * 128:(cb + 1) * 128, :].T
    m2b = np.ascontiguousarray(np.stack([m2_b[:128], m2_b[128:]], axis=1),
                               dtype=np.float32)

    return dict(
        w9T=w9T.astype(np.float16), bias_total=bias_total, WmT=WmT, bm4=bm4,
        m1T=m1T, m1b=m1_b.reshape(64, 1).astype(np.float32),
        m2T=m2T, m2b=m2b,
        iotaRow=np.tile(np.arange(128, dtype=np.float32)[None, :], (128, 1)),
        iotaCol=np.arange(128, dtype=np.float32).reshape(128, 1),
        I128=np.eye(128, dtype=np.float32),
        ones_col=np.ones((128, 1), np.float32),
    )


def _build_program():
    from contextlib import ExitStack
    import concourse.bass as bass
    import concourse.bacc as bacc
    import concourse.tile as tile
    from concourse import mybir

    f32 = mybir.dt.float32
    f32r = mybir.dt.float32r
    i16 = mybir.dt.int16
    i32 = mybir.dt.int32
    Alu = mybir.AluOpType
    Act = mybir.ActivationFunctionType

    nc = bacc.Bacc("TRN2", target_bir_lowering=False, debug=False)

    rgb_in = nc.dram_tensor("rgb_in", [C, HW], f32, kind="ExternalInput")
    tir_in = nc.dram_tensor("tir_in", [C, HW], f32, kind="ExternalInput")
    bf16 = mybir.dt.float16
    params = {n: nc.dram_tensor(n, s, bf16 if n == "w9T" else f32,
                                kind="ExternalInput")
              for n, s in _PARAM_SHAPES.items()}
    rgb_out = nc.dram_tensor("rgb_out", [C, HW], f32, kind="ExternalOutput")
    tir_out = nc.dram_tensor("tir_out", [C, HW], f32, kind="ExternalOutput")
    q_out = nc.dram_tensor("q_out", [H, W], f32, kind="ExternalOutput")

    pm = [nc.dram_tensor(f"pm{m}", [HW, C], f32) for m in range(2)]
    feat_in = [rgb_in, tir_in]
    al_out = [rgb_out, tir_out]

    ctx = ExitStack()
    with ctx:
        tc = ctx.enter_context(tile.TileContext(nc))
        sb = ctx.enter_context(tc.tile_pool(name="consts", bufs=1))
        p_sb = {n: sb.tile(s, bf16 if n == "w9T" else f32, tag=n, name=n)
                for n, s in _PARAM_SHAPES.items()}
        for n in _PARAM_SHAPES:
            nc.sync.dma_start(out=p_sb[n], in_=params[n].ap())
        I128_t = p_sb["I128"]
        ones_col = p_sb["ones_col"]

        # padded CM slabs [128, 10 rows, RP], blocks = rgb0,rgb1,tir0,tir1
        slab = [[sb.tile([128, 10, RP], f32, tag=f"slab{b}_{par}", name=f"slab{b}_{par}")
                 for par in range(2)] for b in range(4)]
        slab_bf = [[sb.tile([128, 10, RP], bf16, tag=f"slabbf{b}_{par}", name=f"slabbf{b}_{par}")
                    for par in range(2)] for b in range(4)]
        for b in range(4):
            for par in range(2):
                nc.vector.memset(slab[b][par], 0.0)
                nc.vector.memset(slab_bf[b][par], 0.0)

        wtiles = [[sb.tile([128, 128], f32, tag=f"w{k}_{m}", name=f"w{k}_{m}") for k in range(4)]
                  for m in range(2)]
        btop = [sb.tile([128, 128, 8], i16, tag=f"btop{m}", name=f"btop{m}") for m in range(2)]
        bbot = [sb.tile([128, 128, 8], i16, tag=f"bbot{m}", name=f"bbot{m}") for m in range(2)]
        mains_sb = sb.tile([128, 128, 4], f32, tag="mains", name="mains")
        pooled_sb = sb.tile([128, 4], f32, tag="pooled", name="pooled")
        m_sb = sb.tile([64, 1], f32, tag="m_hid", name="m_hid")
        mod_sb = sb.tile([128, 2], f32, tag="mod", name="mod")
        qrow_sb = sb.tile([128, 128], f32, tag="qrow", name="qrow")

        psacc = ctx.enter_context(tc.tile_pool(name="psacc", bufs=1, space="PSUM"))
        ps_mains = psacc.tile([128, 128, 4], f32, tag="ps_mains", name="ps_mains")
        ps_gap = psacc.tile([128, 4], f32, tag="ps_gap", name="ps_gap")
        ps_q = psacc.tile([128, 128], f32, tag="ps_q", name="ps_q")

        smallp = ctx.enter_context(tc.tile_pool(name="small", bufs=2))

        # =================== PHASE 1 ===================
        nchunks = H // NRC
        with tc.tile_pool(name="psum_p1", bufs=2, space="PSUM") as psum1, \
             tc.tile_pool(name="x1", bufs=3) as x1p, \
             tc.tile_pool(name="pmstage", bufs=3) as pmstage:
            for t in range(nchunks):
                r0 = t * NRC
                par = t % 2
                lo = max(r0 - 1, 0)
                hi = min(r0 + NRC + 1, H)
                s0 = lo - (r0 - 1)
                for b in range(4):
                    mi, cb = divmod(b, 2)
                    src = feat_in[mi].ap()[cb * 128:(cb + 1) * 128, lo * W:hi * W]
                    nc.sync.dma_start(
                        out=slab[b][par][:, s0:s0 + (hi - lo), 1:1 + W], in_=src)
                    if t == 0:
                        nc.vector.memset(slab[b][par][:, 0, :], 0.0)
                    if t == nchunks - 1:
                        nc.vector.memset(slab[b][par][:, 9, :], 0.0)
                    nc.vector.tensor_copy(slab_bf[b][par], slab[b][par])

                for hh in range(2):
                    ps_x1 = psum1.tile([64, 512], f32, tag="ps_x1", name="ps_x1")
                    for b in range(4):
                        for k in range(9):
                            dy, dx = k // 3 - 1, k % 3 - 1
                            srow = hh * 4 + dy + 1
                            rhs = slab_bf[b][par][:, srow:srow + 4,
                                                       1 + dx:1 + dx + W]
                            lhsT = p_sb["w9T"][:, (b * 9 + k) * 64:(b * 9 + k + 1) * 64]
                            nc.tensor.matmul(
                                ps_x1, lhsT, rhs,
                                start=(b == 0 and k == 0),
                                stop=(b == 3 and k == 8))
                    x1_t = x1p.tile([64, 512], f32, tag="x1", name="x1")
                    nc.scalar.activation(x1_t, ps_x1, Act.Relu,
                                         bias=p_sb["bias_total"][:, 0:1], scale=1.0)
                    for jj in range(4):
                        jrow = r0 + hh * 4 + jj
                        nc.tensor.matmul(ps_mains[:, jrow, :],
                                         x1_t[:, jj * 128:(jj + 1) * 128],
                                         p_sb["WmT"], start=True, stop=True,
                                         skip_group_check=True)

                for jj in range(0, NRC, 2):
                    for mi in range(2):
                        ps_t = psum1.tile([128, 2, 2, 128], f32, tag="ps_trans", name="ps_trans")
                        pm_sb = pmstage.tile([128, 2, 2, 128], f32, tag="pmsb", name="pmsb")
                        for kk in range(2):
                            slot = jj + kk + 1
                            for cb in range(2):
                                nc.tensor.transpose(
                                    ps_t[:, kk, cb, :],
                                    slab[mi * 2 + cb][par][:, slot, 1:1 + W],
                                    I128_t)
                        nc.scalar.copy(pm_sb, ps_t)
                        for kk in range(2):
                            for cb in range(2):
                                nc.tensor.matmul(
                                    ps_gap[:, mi * 2 + cb:mi * 2 + cb + 1],
                                    pm_sb[:, kk, cb, :],
                                    ones_col,
                                    start=(t == 0 and jj == 0 and kk == 0),
                                    stop=(t == nchunks - 1 and jj == NRC - 2
                                          and kk == 1),
                                    skip_group_check=True)
                        row = r0 + jj
                        dst = bass.AP(tensor=pm[mi].ap().tensor,
                                      offset=row * W * C,
                                      ap=[[C, 128], [W * C, 2], [128, 2],
                                          [1, 128]])
                        nc.sync.dma_start(out=dst, in_=pm_sb)

        # =================== PHASE 1.5 ===================
        with tc.tile_pool(name="psum_mid", bufs=2, space="PSUM") as psumm:
            nc.scalar.copy(pooled_sb, ps_gap)
            ps_m = psumm.tile([64, 1], f32, tag="ps_small", name="ps_small")
            for b in range(4):
                nc.tensor.matmul(ps_m, p_sb["m1T"][:, b * 64:(b + 1) * 64],
                                 pooled_sb[:, b:b + 1],
                                 start=(b == 0), stop=(b == 3))
            nc.scalar.activation(m_sb, ps_m, Act.Relu,
                                 bias=p_sb["m1b"][:, 0:1], scale=1.0 / HW)
            ps_mod = psumm.tile([128, 2], f32, tag="ps_small", name="ps_small")
            for cb in range(2):
                nc.tensor.matmul(ps_mod[:, cb:cb + 1],
                                 p_sb["m2T"][:, cb * 128:(cb + 1) * 128],
                                 m_sb, start=(cb == 0), stop=(cb == 1),
                                 skip_group_check=True)
            for cb in range(2):
                nc.scalar.activation(mod_sb[:, cb:cb + 1], ps_mod[:, cb:cb + 1],
                                     Act.Sigmoid, bias=p_sb["m2b"][:, cb:cb + 1],
                                     scale=1.0)

            nc.vector.tensor_copy(mains_sb, ps_mains)

            for mi in range(2):
                mxv = mains_sb[:, :, 2 * mi]
                myv = mains_sb[:, :, 2 * mi + 1]
                ix = smallp.tile([128, 128], f32, tag="ix", name="ix")
                iy = smallp.tile([128, 128], f32, tag="iy", name="iy")
                nc.vector.tensor_scalar(ix, mxv, p_sb["iotaCol"][:, 0:1],
                                        p_sb["bm4"][:, 2 * mi:2 * mi + 1],
                                        op0=Alu.add, op1=Alu.add)
                nc.vector.tensor_scalar(ix, ix, 0.0, 127.0,
                                        op0=Alu.max, op1=Alu.min)
                nc.vector.scalar_tensor_tensor(
                    iy, myv, p_sb["bm4"][:, 2 * mi + 1:2 * mi + 2],
                    p_sb["iotaRow"], op0=Alu.add, op1=Alu.add)
                nc.vector.tensor_scalar(iy, iy, 0.0, 127.0,
                                        op0=Alu.max, op1=Alu.min)
                xb = smallp.tile([128, 128], f32, tag="xb", name="xb")
                yb = smallp.tile([128, 128], f32, tag="yb", name="yb")
                wx = smallp.tile([128, 128], f32, tag="wx", name="wx")
                wy = smallp.tile([128, 128], f32, tag="wy", name="wy")
                tmp = smallp.tile([128, 128], f32, tag="ctmp", name="ctmp")
                tmpi = smallp.tile([128, 128], i32, tag="ctmpi", name="ctmpi")
                # floor via int cast (any rounding mode): y=cast(x); y-=(y>x)
                for src_t, base_t, frac_t in ((ix, xb, wx), (iy, yb, wy)):
                    nc.vector.tensor_copy(tmpi, src_t)
                    nc.vector.tensor_copy(base_t, tmpi)
                    nc.vector.tensor_tensor(tmp, base_t, src_t, op=Alu.is_gt)
                    nc.vector.tensor_tensor(base_t, base_t, tmp, op=Alu.subtract)
                    nc.vector.tensor_scalar(base_t, base_t, 126.0, None,
                                            op0=Alu.min)
                    nc.vector.tensor_tensor(frac_t, src_t, base_t,
                                            op=Alu.subtract)
                u = smallp.tile([128, 128], f32, tag="u", name="u")
                v = smallp.tile([128, 128], f32, tag="v", name="v")
                nc.vector.tensor_scalar(u, wx, -1.0, 1.0, op0=Alu.mult, op1=Alu.add)
                nc.vector.tensor_scalar(v, wy, -1.0, 1.0, op0=Alu.mult, op1=Alu.add)
                nc.vector.tensor_tensor(wtiles[mi][0], u, v, op=Alu.mult)
                nc.vector.tensor_tensor(wtiles[mi][1], wx, v, op=Alu.mult)
                nc.vector.tensor_tensor(wtiles[mi][2], u, wy, op=Alu.mult)
                nc.vector.tensor_tensor(wtiles[mi][3], wx, wy, op=Alu.mult)

                idxA = smallp.tile([128, 128], f32, tag="idxA", name="idxA")
                nc.vector.scalar_tensor_tensor(idxA, yb, 128.0, xb,
                                               op0=Alu.mult, op1=Alu.add)
                ps_sw = psumm.tile([128, 128], f32, tag="ps_small", name="ps_small")
                nc.tensor.transpose(ps_sw, idxA, I128_t)
                s1 = smallp.tile([128, 128], f32, tag="s1", name="s1")
                nc.vector.tensor_copy(s1, ps_sw)
                rep = smallp.tile([128, 128], f32, tag="rep", name="rep")
                for g in range(8):
                    for r in range(8):
                        nc.vector.tensor_copy(rep[:, r * 16:(r + 1) * 16],
                                              s1[:, g * 16:(g + 1) * 16])
                    ps_b = psumm.tile([128, 128], f32, tag="ps_small", name="ps_small")
                    nc.tensor.transpose(ps_b, rep, I128_t)
                    nc.vector.tensor_copy(btop[mi][:, :, g], ps_b)
                nc.vector.tensor_scalar(bbot[mi], btop[mi], 128, None,
                                        op0=Alu.add)

        # =================== PHASE 2 ===================
        nbatches = HW // NBPX
        rpb = NBPX // W
        with tc.tile_pool(name="psum_p2", bufs=2, space="PSUM") as psum2, \
             tc.tile_pool(name="gbuf", bufs=2) as gbufp, \
             tc.tile_pool(name="comb", bufs=3) as combp, \
             tc.tile_pool(name="outcm", bufs=3) as outp:
            for bt in range(nbatches):
                gb = {}
                for mi in range(2):
                    in_ap = bass.AP(tensor=pm[mi].ap().tensor, offset=0,
                                    ap=[[C, HW - 1], [1, 2 * C]])
                    for si, bsrc in enumerate((btop, bbot)):
                        g_t = gbufp.tile([128, rpb, 2 * C], f32, tag=f"g{mi}{si}", name=f"g{mi}{si}")
                        nc.gpsimd.dma_gather(
                            out_ap=g_t, in_ap=in_ap,
                            idxs_ap=bsrc[mi][:, bt * rpb:(bt + 1) * rpb, :],
                            num_idxs=NBPX, num_idxs_reg=NBPX,
                            elem_size=2 * C, elem_step=C)
                        gb[(mi, si)] = g_t

                for jj in range(0, rpb, 2):
                    cm_sb = {}
                    for mi in range(2):
                        ps_o = psum2.tile([128, 2, 2, 128], f32, tag="ps_o", name="ps_o")
                        for kk in range(2):
                            J = jj + kk
                            jrow = bt * rpb + J
                            o = combp.tile([128, 2, 128], f32, tag="o", name="o")
                            gt, gbo = gb[(mi, 0)], gb[(mi, 1)]
                            wc = [w[:, jrow:jrow + 1] for w in wtiles[mi]]
                            nc.vector.tensor_scalar(
                                o, gt[:, J, 0:256], wc[0], None, op0=Alu.mult)
                            nc.vector.scalar_tensor_tensor(
                                o, gt[:, J, 256:512], wc[1], o,
                                op0=Alu.mult, op1=Alu.add)
                            nc.vector.scalar_tensor_tensor(
                                o, gbo[:, J, 0:256], wc[2], o,
                                op0=Alu.mult, op1=Alu.add)
                            nc.vector.scalar_tensor_tensor(
                                o, gbo[:, J, 256:512], wc[3], o,
                                op0=Alu.mult, op1=Alu.add)
                            for cb in range(2):
                                nc.tensor.transpose(ps_o[:, cb, kk, :],
                                                    o[:, cb, :], I128_t)
                        oc = outp.tile([128, 2, 2, 128], f32, tag=f"oc{mi}", name=f"oc{mi}")
                        for cb in range(2):
                            nc.scalar.activation(
                                oc[:, cb], ps_o[:, cb], Act.Copy,
                                scale=mod_sb[:, cb:cb + 1])
                        cm_sb[mi] = oc
                        row = bt * rpb + jj
                        for cb in range(2):
                            dst = bass.AP(
                                tensor=al_out[mi].ap().tensor,
                                offset=cb * 128 * HW + row * W,
                                ap=[[HW, 128], [W, 2], [1, W]])
                            nc.sync.dma_start(out=dst, in_=oc[:, cb])
                    dts = []
                    for cb in range(2):
                        d = combp.tile([128, 2, 128], f32, tag=f"qd{cb}",
                                       name=f"qd{cb}")
                        nc.vector.tensor_tensor(d, cm_sb[0][:, cb],
                                                cm_sb[1][:, cb], op=Alu.subtract)
                        nc.vector.tensor_scalar(
                            d.bitcast(i32), d.bitcast(i32), 0x7FFFFFFF, None,
                            op0=Alu.bitwise_and)
                        dts.append(d)
                    for kk in range(2):
                        jrow = bt * rpb + jj + kk
                        for cb in range(2):
                            nc.tensor.matmul(
                                ps_q[:, jrow:jrow + 1], dts[cb][:, kk, :],
                                ones_col, start=(cb == 0), stop=(cb == 1),
                                skip_group_check=True)

            # quality finish
            nc.scalar.activation(qrow_sb, ps_q, Act.Sigmoid,
                                 bias=1.0, scale=-1.0 / 256.0)
            ps_qt = psum2.tile([128, 128], f32, tag="ps_o", name="ps_o")
            nc.tensor.transpose(ps_qt, qrow_sb, I128_t)
            qf = smallp.tile([128, 128], f32, tag="qf", name="qf")
            nc.vector.tensor_copy(qf, ps_qt)
            nc.sync.dma_start(out=q_out.ap(), in_=qf)

    nc.compile()
    return nc


def kernel(**inputs):
    global LAST_EXEC_NS
    from concourse.bass_utils import run_bass_kernel_spmd

    fw = _fold_weights(inputs)
    rgb = np.ascontiguousarray(np.asarray(inputs['rgb_feat'], np.float32))
    tir = np.ascontiguousarray(np.asarray(inputs['tir_feat'], np.float32))
    B = rgb.shape[0]

    if 'nc' not in _CACHE:
        _CACHE['nc'] = _build_program()
    nc = _CACHE['nc']

    in_maps = []
    for b in range(B):
        m = {'rgb_in': np.ascontiguousarray(rgb[b].reshape(C, HW)),
             'tir_in': np.ascontiguousarray(tir[b].reshape(C, HW))}
        for k, v in fw.items():
            m[k] = np.ascontiguousarray(v)
        in_maps.append(m)

    t0 = time.monotonic()
    res = run_bass_kernel_spmd(nc, in_maps, list(range(B)))
    t1 = time.monotonic()
    LAST_EXEC_NS = res.exec_time_ns if res.exec_time_ns else int((t1 - t0) * 1e9)

    rgb_al = np.stack([np.asarray(res.results[b]['rgb_out']).reshape(C, H, W)
                       for b in range(B)])
    tir_al = np.stack([np.asarray(res.results[b]['tir_out']).reshape(C, H, W)
                       for b in range(B)])
    quality = np.stack([np.asarray(res.results[b]['q_out']).reshape(1, H, W)
                        for b in range(B)])
    return rgb_al, tir_al, quality
